# revision 19
# baseline (speedup 1.0000x reference)
# Trainium2 Bass kernel for nn_CalcDeformation (scatter 1024 betas onto a
# regular 32x32 stride-8 grid in a 256x256 image, depthwise-conv with a
# shared 31x31 kernel, 2 channels, batch 128 -> output [128, 65536, 2]).
#
# Because the scatter centers form a regular stride-8 grid, scatter+conv is
# a transposed convolution. Writing output rows R = 8*qr + pr, each output
# row only sees 4 consecutive control-grid rows gr = qr + e:
#   out[b,ch,R,C] = sum_{gr,gc} beta[b,gr,gc,ch] * K[rk-R+15, ck-C+15],
#   (rk,ck) = (8gr+4, 8gc+4);  kernel row = 8e-pr+19, col = 8gc-C+19,
#   e in {-2..1} for pr 0..3 (group g=0), e in {-1..2} for pr 4..7 (g=1).
#
# This becomes one 128-contraction matmul per (batch-chunk, group, channel,
# row-phase pair):   out[(b,qr), (pr,C)] = L^T @ W   with
#   L[(jj,gc'), (b,qr)] = beta[b, qr+e_base(g)+jj, 31-gc', ch]      (lhsT)
#   W[(jj,gc'), (pr,C)] = K[8jj+3-pr+4g, 8*(31-gc')-C+19]   (0 if invalid)
# Operands are bf16 (tolerance is 2e-2; bf16 keeps rel err ~5e-3).
#
# The kernel is output-bandwidth bound, so everything is organized around
# the store stream (HBM-per-core limit ~358 GB/s):
#  - the OUTPUT IS STORED AS BF16 and widened to f32 on the host, halving
#    the dominant store traffic (8.4 -> 4.2 MB per core);
#  - the output is written as two CHANNEL PLANES and the host interleaves
#    channels back: plane copies write contiguous bf16, which runs
#    DVE/ACT at ~1 elem/cycle (the on-chip stride-2 interleave measured
#    ~half rate - partial-word writes), and PSUM->SBUF copies stay
#    FD=512 single-bank (multi-bank copy APs measured to starve PE);
#  - stores go out at (chunk, g, ch) 256 KB granularity (2-copy
#    dependency keeps the queue fed at ~370 GB/s; chunk 0 at 128 KB
#    single-copy granularity to start the drain early).
# Measured on the 8-core SPMD run: 37.7 us (f32 interleaved baseline) ->
# 30.5 us; ~7.3 us of that is the fixed NRT postamble (per-engine
# semaphore-file clears), which is not controllable from the kernel.
# (Tried and rejected: on-chip L expansion - partition-base-shifted
# copies hit a ~2.7us microcoded path; fused FD=1024+ copies - slower
# per element and starve PE; walrus --max-sem-num - does not shrink the
# NRT postamble.)
#
# Sharding: pure batch data parallel, 16 batches per core on 8 cores.
# Host-side prep is pure indexing; all arithmetic runs on device.
import os

import ml_dtypes
import numpy as np

import concourse.bass as bass
import concourse.bacc as bacc
import concourse.mybir as mybir
import concourse.tile as tile
from concourse.bass_utils import run_bass_kernel_spmd

F32 = mybir.dt.float32
BF16 = mybir.dt.bfloat16
NP_BF16 = ml_dtypes.bfloat16

N_CORES = 8
BATCH = 128
B_L = BATCH // N_CORES
KS = 31
IMG = 256
N_OUT = B_L * IMG * IMG * 2


def _ap(t, off, pat):
    return bass.AP(tensor=t.ap().tensor, offset=off, ap=[list(p) for p in pat])


def _host_prepare_w(kern):
    """kern [31,31] -> w2 [128, 2048] bf16:
    w2[jj*32+gc', (g, pr, C)] = K[8jj+3-pr+4g, 8*(31-gc')-C+19] or 0."""
    kp = np.zeros((KS, 504), np.float32)
    kp[:, 237:268] = kern[:, ::-1]
    swv = np.lib.stride_tricks.sliding_window_view(kp, 256, axis=1)
    w2 = np.zeros((4, 32, 2, 4, 256), np.float32)  # [jj, gc', g, pr, C]
    cols = 8 * np.arange(32)
    for g in range(2):
        for jj in range(4):
            for pr in range(4):
                kr = 8 * jj + 3 - pr + 4 * g
                if 0 <= kr <= 30:
                    w2[jj, :, g, pr, :] = swv[kr, cols, :]
    return w2.reshape(128, 2048).astype(NP_BF16)


def _host_prepare_p(betas_core):
    """betas_core [B_L,1024,2] (k = gr*32+gc) -> P [32, 1152] bf16 with
    free dim (chunk 4, ch 2, b 4, gp 36):
    P[gc', chunk, ch, b, gp] = beta[chunk*4+b, gp-2, 31-gc', ch] (0 pad)."""
    bg = betas_core.reshape(B_L, 32, 32, 2)
    p = np.zeros((32, 4, 2, 4, 36), np.float32)
    src = bg[:, :, ::-1, :]  # [b16, gr, gc' reversed, ch]
    for chunk in range(4):
        for b in range(4):
            p[:, chunk, :, b, 2:34] = src[chunk * 4 + b].transpose(1, 2, 0)
    return p.reshape(32, 1152).astype(NP_BF16)


def _host_prepare_l(betas_core):
    """Reference L layout (used by experiments): [128, 2048] bf16,
    l5[jj*32+gc', (chunk, t=(ch*2+g), b_local, qr)]."""
    bg = betas_core.reshape(B_L, 32, 32, 2)
    l4 = np.zeros((4, 32, 4, B_L, 32), np.float32)  # [jj, gc', t, b, qr]
    for ch in range(2):
        for g in range(2):
            e_base = -2 + g
            t = ch * 2 + g
            for jj in range(4):
                e = e_base + jj
                lo = max(0, -e)
                cnt = 32 - abs(e)
                l4[jj, :, t, :, lo:lo + cnt] = (
                    bg[:, lo + e:lo + e + cnt, ::-1, ch].transpose(2, 0, 1)
                )
    l5 = l4.reshape(128, 4, 4, 4, 32).transpose(0, 2, 1, 3, 4)
    return np.ascontiguousarray(l5).reshape(128, 2048).astype(NP_BF16)


def _build_nc():
    nc = bacc.Bacc("TRN2", target_bir_lowering=False, debug=False,
                   num_devices=N_CORES)
    l4d = nc.dram_tensor("l4", [128 * 2048], BF16, kind="ExternalInput")
    w2d = nc.dram_tensor("w2", [128 * 2048], BF16, kind="ExternalInput")
    # Output is stored as bf16 (tolerance is 2e-2; bf16 keeps rel err ~4e-3)
    # and widened to f32 on the host: halves the dominant store traffic
    # (8.4 MB -> 4.2 MB per core).
    out = nc.dram_tensor("out", [N_OUT], BF16, kind="ExternalOutput")

    with tile.TileContext(nc) as tc:
        with (
            tc.tile_pool(name="wp", bufs=1) as wp,
            tc.tile_pool(name="lp", bufs=1) as lp,
            tc.tile_pool(name="s0p", bufs=6) as s0p,
            tc.tile_pool(name="pp", bufs=8, space="PSUM") as pp,
        ):
            W = wp.tile([128, 2048], BF16, tag="w")
            L = lp.tile([128, 2048], BF16, tag="l")

            def load_w_cols(c0, c1, eng):
                eng.dma_start(
                    out=W[:, c0:c1],
                    in_=_ap(w2d, c0, [[2048, 128], [1, c1 - c0]]))

            def load_l_chunks(c0, c1, eng):
                off = c0 * 512
                eng.dma_start(
                    out=L[:, off:c1 * 512],
                    in_=_ap(l4d, off, [[2048, 128], [1, (c1 - c0) * 512]]))

            # Prologue loads.  Group (0, g=0) needs W cols 0:1024 + L chunk0
            # on the critical path; load those first on the two HWDGE queues
            # (sync + scalar) in parallel.  The rest streams behind: W g=1
            # on scalar, L chunks 1-3 on the gpsimd SWDGE queue (chunk 1
            # separate so its completion fires early).  Batched loads keep
            # per-partition descriptors at 1-4 KB.
            load_w_cols(0, 512, nc.sync)
            load_l_chunks(0, 1, nc.scalar)
            load_w_cols(512, 1024, nc.sync)
            load_w_cols(1024, 2048, nc.scalar)
            load_l_chunks(1, 2, nc.gpsimd)
            load_l_chunks(2, 4, nc.gpsimd)

            # Output is written as two CHANNEL PLANES [ch, b, 256, 256]
            # (host interleaves ch back to [b, pix, 2]): plane copies write
            # CONTIGUOUS bf16, which runs the engines at full rate (the
            # stride-2 on-chip interleave measured at half rate on ACT —
            # partial-word writes).  Per piece (chunk, g, pr2): one matmul
            # per channel into its own 1-bank PSUM tile (PE keeps
            # streaming; multi-bank copies measured to starve PE), DVE
            # copies ch0, ACT copies ch1.
            def mm(chunk, g, pr2, ch):
                t = ch * 2 + g
                Pm = pp.tile([128, 512], F32, tag="psum")
                nc.tensor.matmul(
                    Pm[:],
                    lhsT=L[:, chunk * 512 + t * 128:
                           chunk * 512 + (t + 1) * 128],
                    rhs=W[:, g * 1024 + pr2 * 512:
                          g * 1024 + (pr2 + 1) * 512],
                    start=True, stop=True,
                )
                return Pm

            def copy(dst, Pm, ch):
                if ch == 0:
                    nc.vector.tensor_copy(dst, Pm[:])
                else:
                    nc.scalar.copy(dst, Pm[:])

            # plane address: ch*1048576 + b*65536 + R*256 + C,
            # R = 8*qr + 4g + pr'  (partition = (b_in_chunk, qr)).
            # Chunk 0 goes out as 128 KB single-copy stores so the drain
            # starts as early as possible (the queue is empty then, so the
            # 1 KB descriptors don't matter).
            for g in range(2):
                for pr2 in range(2):
                    for ch in range(2):
                        Pm = mm(0, g, pr2, ch)
                        S1 = s0p.tile([128, 512], BF16, tag="s0s")
                        copy(S1[:], Pm, ch)
                        nc.sync.dma_start(
                            out=_ap(out,
                                    ch * 1048576 + g * 1024 + pr2 * 512,
                                    [[65536, 4], [2048, 32], [1, 512]]),
                            in_=S1[:],
                        )

            # Chunks 1-3: 256 KB stores at (chunk, g, ch) granularity: each
            # depends on only TWO same-engine copies, so the store queue is
            # fed at a steady ~370 GB/s with no per-chunk dependency
            # bursts.  Per-partition runs are 4 rows = 2 KB descriptors.
            for chunk in range(1, 4):
                for g in range(2):
                    Ps = [[mm(chunk, g, pr2, ch) for ch in range(2)]
                          for pr2 in range(2)]
                    for ch in range(2):
                        S = s0p.tile([128, 1024], BF16, tag="s0")
                        for pr2 in range(2):
                            copy(S[:, pr2 * 512:(pr2 + 1) * 512],
                                 Ps[pr2][ch], ch)
                        nc.sync.dma_start(
                            out=_ap(out,
                                    ch * 1048576 + chunk * 262144 + g * 1024,
                                    [[65536, 4], [2048, 32], [1, 1024]]),
                            in_=S[:],
                        )
    nc.compile()
    return nc


_NC_CACHE = None


def _get_nc():
    global _NC_CACHE
    if _NC_CACHE is None:
        _NC_CACHE = _build_nc()
    return _NC_CACHE


def _grid_permute(betas, g_centers):
    """Reorder betas so that k = gr*32 + gc (row-major regular grid)."""
    rows = g_centers[:, 0].astype(np.int64)
    cols = g_centers[:, 1].astype(np.int64)
    gr, gc = (rows - 4) // 8, (cols - 4) // 8
    ok = (np.array_equal(rows, gr * 8 + 4) and np.array_equal(cols, gc * 8 + 4)
          and gr.min() >= 0 and gr.max() < 32
          and gc.min() >= 0 and gc.max() < 32)
    if not ok:
        raise NotImplementedError("g_centers is not the regular 32x32 grid")
    gidx = gr * 32 + gc
    if len(np.unique(gidx)) != 1024:
        raise NotImplementedError("duplicate g_centers")
    bg = np.empty_like(betas)
    bg[:, gidx, :] = betas
    return bg


LAST_RESULTS = None  # BassKernelResults of the most recent run (for test.py)


def kernel(betas, kernel, g_centers):
    betas = np.ascontiguousarray(np.asarray(betas, dtype=np.float32))
    kern = np.asarray(kernel, dtype=np.float32)
    g_centers = np.asarray(g_centers)
    assert betas.shape == (BATCH, 1024, 2) and kern.shape == (KS, KS)

    bg = _grid_permute(betas, g_centers)
    w2 = _host_prepare_w(kern).reshape(-1)
    in_maps = [
        {"l4": _host_prepare_l(bg[c * B_L:(c + 1) * B_L]).reshape(-1),
         "w2": w2}
        for c in range(N_CORES)
    ]

    nc = _get_nc()
    trace = os.environ.get("DEFORM_TRACE", "") == "1"
    res = run_bass_kernel_spmd(nc, in_maps, core_ids=list(range(N_CORES)),
                               trace=trace)
    global LAST_RESULTS
    LAST_RESULTS = res

    out = np.empty((BATCH, IMG * IMG, 2), np.float32)
    for c in range(N_CORES):
        planes = res.results[c]["out"].reshape(2, B_L, IMG * IMG)
        # bf16 -> f32 widening + channel de-interleave on host
        out[c * B_L:(c + 1) * B_L, :, 0] = planes[0]
        out[c * B_L:(c + 1) * B_L, :, 1] = planes[1]
    return out



# revision 23
# speedup vs baseline: 1.1926x; 1.1926x over previous
# Trainium2 Bass kernel for nn_CalcDeformation (scatter 1024 betas onto a
# regular 32x32 stride-8 grid in a 256x256 image, depthwise-conv with a
# shared 31x31 kernel, 2 channels, batch 128 -> output [128, 65536, 2]).
#
# Because the scatter centers form a regular stride-8 grid, scatter+conv is
# a transposed convolution. Writing output rows R = 8*qr + pr, each output
# row only sees 4 consecutive control-grid rows gr = qr + e:
#   out[b,ch,R,C] = sum_{gr,gc} beta[b,gr,gc,ch] * K[rk-R+15, ck-C+15],
#   (rk,ck) = (8gr+4, 8gc+4);  kernel row = 8e-pr+19, col = 8gc-C+19,
#   e in {-2..1} for pr 0..3 (group g=0), e in {-1..2} for pr 4..7 (g=1).
#
# This becomes one 128-contraction matmul per (batch-chunk, group, channel,
# row-phase pair):   out[(b,qr), (pr,C)] = L^T @ W   with
#   L[(jj,gc'), (b,qr)] = beta[b, qr+e_base(g)+jj, 31-gc', ch]      (lhsT)
#   W[(jj,gc'), (pr,C)] = K[8jj+3-pr+4g, 8*(31-gc')-C+19]   (0 if invalid)
# Operands are bf16 (tolerance is 2e-2; bf16 keeps rel err ~5e-3).
#
# The kernel is output-bandwidth bound, so everything is organized around
# the store stream (HBM-per-core limit ~358 GB/s):
#  - the OUTPUT IS STORED AS BF16 and widened to f32 on the host, halving
#    the dominant store traffic (8.4 -> 4.2 MB per core);
#  - the output is written as two CHANNEL PLANES and the host interleaves
#    channels back: plane copies write contiguous bf16, which runs
#    DVE/ACT at ~1 elem/cycle (the on-chip stride-2 interleave measured
#    ~half rate - partial-word writes), and PSUM->SBUF copies stay
#    FD=512 single-bank (multi-bank copy APs measured to starve PE);
#  - stores go out at (chunk, g, ch) 256 KB granularity (2-copy
#    dependency keeps the queue fed at ~370 GB/s with 2 KB descriptors).
# Measured on the 8-core SPMD run: 37.7 us (f32 interleaved baseline) ->
# 30.5 us; ~7.3 us of that is the fixed NRT postamble (per-engine
# semaphore-file clears), which is not controllable from the kernel.
# (Tried and rejected: on-chip L expansion - partition-base-shifted
# copies hit a ~2.7us microcoded path; fused FD=1024+ copies - slower
# per element and starve PE; walrus --max-sem-num - does not shrink the
# NRT postamble.)
#
# Sharding: pure batch data parallel, 16 batches per core on 8 cores.
# Host-side prep is pure indexing; all arithmetic runs on device.
import os

import ml_dtypes
import numpy as np

import concourse.bass as bass
import concourse.bacc as bacc
import concourse.mybir as mybir
import concourse.tile as tile
from concourse.bass_utils import run_bass_kernel_spmd

F32 = mybir.dt.float32
BF16 = mybir.dt.bfloat16
NP_BF16 = ml_dtypes.bfloat16

N_CORES = 8
BATCH = 128
B_L = BATCH // N_CORES
KS = 31
IMG = 256
N_OUT = B_L * IMG * IMG * 2


def _ap(t, off, pat):
    return bass.AP(tensor=t.ap().tensor, offset=off, ap=[list(p) for p in pat])


def _host_prepare_w(kern):
    """kern [31,31] -> w2 [128, 2048] bf16:
    w2[jj*32+gc', (g, pr, C)] = K[8jj+3-pr+4g, 8*(31-gc')-C+19] or 0."""
    kp = np.zeros((KS, 504), np.float32)
    kp[:, 237:268] = kern[:, ::-1]
    swv = np.lib.stride_tricks.sliding_window_view(kp, 256, axis=1)
    w2 = np.zeros((4, 32, 2, 4, 256), np.float32)  # [jj, gc', g, pr, C]
    cols = 8 * np.arange(32)
    for g in range(2):
        for jj in range(4):
            for pr in range(4):
                kr = 8 * jj + 3 - pr + 4 * g
                if 0 <= kr <= 30:
                    w2[jj, :, g, pr, :] = swv[kr, cols, :]
    return w2.reshape(128, 2048).astype(NP_BF16)


def _host_prepare_p(betas_core):
    """betas_core [B_L,1024,2] (k = gr*32+gc) -> P [32, 1152] bf16 with
    free dim (chunk 4, ch 2, b 4, gp 36):
    P[gc', chunk, ch, b, gp] = beta[chunk*4+b, gp-2, 31-gc', ch] (0 pad)."""
    bg = betas_core.reshape(B_L, 32, 32, 2)
    p = np.zeros((32, 4, 2, 4, 36), np.float32)
    src = bg[:, :, ::-1, :]  # [b16, gr, gc' reversed, ch]
    for chunk in range(4):
        for b in range(4):
            p[:, chunk, :, b, 2:34] = src[chunk * 4 + b].transpose(1, 2, 0)
    return p.reshape(32, 1152).astype(NP_BF16)


def _host_prepare_l(betas_core):
    """Reference L layout (used by experiments): [128, 2048] bf16,
    l5[jj*32+gc', (chunk, t=(ch*2+g), b_local, qr)]."""
    bg = betas_core.reshape(B_L, 32, 32, 2)
    l4 = np.zeros((4, 32, 4, B_L, 32), np.float32)  # [jj, gc', t, b, qr]
    for ch in range(2):
        for g in range(2):
            e_base = -2 + g
            t = ch * 2 + g
            for jj in range(4):
                e = e_base + jj
                lo = max(0, -e)
                cnt = 32 - abs(e)
                l4[jj, :, t, :, lo:lo + cnt] = (
                    bg[:, lo + e:lo + e + cnt, ::-1, ch].transpose(2, 0, 1)
                )
    l5 = l4.reshape(128, 4, 4, 4, 32).transpose(0, 2, 1, 3, 4)
    return np.ascontiguousarray(l5).reshape(128, 2048).astype(NP_BF16)


def _build_nc():
    nc = bacc.Bacc("TRN2", target_bir_lowering=False, debug=False,
                   num_devices=N_CORES)
    l4d = nc.dram_tensor("l4", [128 * 2048], BF16, kind="ExternalInput")
    w2d = nc.dram_tensor("w2", [128 * 2048], BF16, kind="ExternalInput")
    # Output is stored as bf16 (tolerance is 2e-2; bf16 keeps rel err ~4e-3)
    # and widened to f32 on the host: halves the dominant store traffic
    # (8.4 MB -> 4.2 MB per core).
    out = nc.dram_tensor("out", [N_OUT], BF16, kind="ExternalOutput")

    with tile.TileContext(nc) as tc:
        with (
            tc.tile_pool(name="wp", bufs=1) as wp,
            tc.tile_pool(name="lp", bufs=1) as lp,
            tc.tile_pool(name="s0p", bufs=12) as s0p,
            tc.tile_pool(name="pp", bufs=8, space="PSUM") as pp,
        ):
            W = wp.tile([128, 2048], BF16, tag="w")
            L = lp.tile([128, 2048], BF16, tag="l")

            def load_w_cols(c0, c1, eng):
                eng.dma_start(
                    out=W[:, c0:c1],
                    in_=_ap(w2d, c0, [[2048, 128], [1, c1 - c0]]))

            def load_l_chunks(c0, c1, eng):
                off = c0 * 512
                eng.dma_start(
                    out=L[:, off:c1 * 512],
                    in_=_ap(l4d, off, [[2048, 128], [1, (c1 - c0) * 512]]))

            # Prologue loads.  Group (0, g=0) needs W cols 0:1024 + L chunk0
            # on the critical path; load those first on the two HWDGE queues
            # (sync + scalar) in parallel.  The rest streams behind: W g=1
            # on scalar, L chunks 1-3 on the gpsimd SWDGE queue (chunk 1
            # separate so its completion fires early).  Batched loads keep
            # per-partition descriptors at 1-4 KB.
            load_w_cols(0, 512, nc.sync)
            load_l_chunks(0, 1, nc.scalar)
            load_w_cols(512, 1024, nc.sync)
            load_w_cols(1024, 2048, nc.scalar)
            load_l_chunks(1, 2, nc.gpsimd)
            load_l_chunks(2, 4, nc.gpsimd)

            # Output is written as two CHANNEL PLANES [ch, b, 256, 256]
            # (host interleaves ch back to [b, pix, 2]): plane copies write
            # CONTIGUOUS bf16, which runs the engines at full rate (the
            # stride-2 on-chip interleave measured at half rate on ACT —
            # partial-word writes).  Per piece (chunk, g, pr2): one matmul
            # per channel into its own 1-bank PSUM tile (PE keeps
            # streaming; multi-bank copies measured to starve PE), DVE
            # copies ch0, ACT copies ch1.
            def mm(chunk, g, pr2, ch):
                t = ch * 2 + g
                Pm = pp.tile([128, 512], F32, tag="psum")
                nc.tensor.matmul(
                    Pm[:],
                    lhsT=L[:, chunk * 512 + t * 128:
                           chunk * 512 + (t + 1) * 128],
                    rhs=W[:, g * 1024 + pr2 * 512:
                          g * 1024 + (pr2 + 1) * 512],
                    start=True, stop=True,
                )
                return Pm

            def copy(dst, Pm, ch):
                if ch == 0:
                    nc.vector.tensor_copy(dst, Pm[:])
                else:
                    nc.scalar.copy(dst, Pm[:])

            # plane address: ch*1048576 + b*65536 + R*256 + C,
            # R = 8*qr + 4g + pr'  (partition = (b_in_chunk, qr)).
            # 256 KB stores at (chunk, g, ch) granularity: each depends on
            # only TWO same-engine copies, so the store queue is fed at a
            # steady ~370 GB/s with no per-chunk dependency bursts.
            # Per-partition runs are 4 rows = 2 KB descriptors.  (128 KB
            # single-copy chunk-0 stores were tried for an earlier drain
            # start and measured MUCH slower - 1 KB descriptors tank the
            # early queue rate.)
            for chunk in range(4):
                for g in range(2):
                    # ch-major matmul order: the two matmuls feeding one
                    # plane store are adjacent, so each store issues right
                    # after its two same-engine copies.
                    Ps = [[None, None], [None, None]]
                    for ch in range(2):
                        for pr2 in range(2):
                            Ps[pr2][ch] = mm(chunk, g, pr2, ch)
                    for ch in range(2):
                        S = s0p.tile([128, 1024], BF16, tag="s0")
                        for pr2 in range(2):
                            copy(S[:, pr2 * 512:(pr2 + 1) * 512],
                                 Ps[pr2][ch], ch)
                        nc.sync.dma_start(
                            out=_ap(out,
                                    ch * 1048576 + chunk * 262144 + g * 1024,
                                    [[65536, 4], [2048, 32], [1, 1024]]),
                            in_=S[:],
                        )
    nc.compile()
    return nc


_NC_CACHE = None


def _get_nc():
    global _NC_CACHE
    if _NC_CACHE is None:
        _NC_CACHE = _build_nc()
    return _NC_CACHE


def _grid_permute(betas, g_centers):
    """Reorder betas so that k = gr*32 + gc (row-major regular grid)."""
    rows = g_centers[:, 0].astype(np.int64)
    cols = g_centers[:, 1].astype(np.int64)
    gr, gc = (rows - 4) // 8, (cols - 4) // 8
    ok = (np.array_equal(rows, gr * 8 + 4) and np.array_equal(cols, gc * 8 + 4)
          and gr.min() >= 0 and gr.max() < 32
          and gc.min() >= 0 and gc.max() < 32)
    if not ok:
        raise NotImplementedError("g_centers is not the regular 32x32 grid")
    gidx = gr * 32 + gc
    if len(np.unique(gidx)) != 1024:
        raise NotImplementedError("duplicate g_centers")
    bg = np.empty_like(betas)
    bg[:, gidx, :] = betas
    return bg


LAST_RESULTS = None  # BassKernelResults of the most recent run (for test.py)


def kernel(betas, kernel, g_centers):
    betas = np.ascontiguousarray(np.asarray(betas, dtype=np.float32))
    kern = np.asarray(kernel, dtype=np.float32)
    g_centers = np.asarray(g_centers)
    assert betas.shape == (BATCH, 1024, 2) and kern.shape == (KS, KS)

    bg = _grid_permute(betas, g_centers)
    w2 = _host_prepare_w(kern).reshape(-1)
    in_maps = [
        {"l4": _host_prepare_l(bg[c * B_L:(c + 1) * B_L]).reshape(-1),
         "w2": w2}
        for c in range(N_CORES)
    ]

    nc = _get_nc()
    trace = os.environ.get("DEFORM_TRACE", "") == "1"
    res = run_bass_kernel_spmd(nc, in_maps, core_ids=list(range(N_CORES)),
                               trace=trace)
    global LAST_RESULTS
    LAST_RESULTS = res

    out = np.empty((BATCH, IMG * IMG, 2), np.float32)
    for c in range(N_CORES):
        planes = res.results[c]["out"].reshape(2, B_L, IMG * IMG)
        # bf16 -> f32 widening + channel de-interleave on host
        out[c * B_L:(c + 1) * B_L, :, 0] = planes[0]
        out[c * B_L:(c + 1) * B_L, :, 1] = planes[1]
    return out

